# revision 1
# baseline (speedup 1.0000x reference)
"""Trainium2 Bass kernel for nn_Neighbor_Mean (gnn message passing).

Math: out[b,s,:] = mean_n( mask[b,s,n] * (T_b[idx[b,s,n]] @ Wn^T) )
 with T_b[v] = pos_table[v] + (h[b][v-1] if v>=1 else 0)   (v in [0, 2049))
Since the mask multiplies matmul outputs and matmul is linear:
 out[b,s,:] = ( (1/N) * sum_n T'_b[idx_eff[b,s,n]] ) @ Wn^T
 where T' has an extra zero row at SINK=2049 and idx_eff = mask ? idx : SINK.

Sharding: data-parallel over batch, one NeuronCore per batch row (B == 8).

Per-core plan:
 - build T' in SBUF as bf16, packed [128 part, 17*128] (row v at partition
   v%128, free chunk v//128) -- the SBUF-source layout of dma_gather
   (tokens_per_rank=128, free_dim_per_rank=256B).
 - fold mask into indices on DVE (select against SINK), emit int16 in the
   16-partition wrapped layout dma_gather wants, replicate to 128 partitions.
 - SBUF->SBUF transposed dma_gather, 512 idx/call (ucode ring ceiling),
   4 SWDGE queues. Gathered tile g[h=128 part, j free], stream
   j = (n, s%16) per call, call c covers s in [16c, 16c+16).
 - PE: per 128-s chunk, PSUM-accumulate 32 matmuls over n:
   psum[s,k] += g_slice[h, s]^T @ (Wn^T * 1/N) (bf16 x bf16 -> f32).
 - copy PSUM->SBUF, DMA out rows (f32).
"""
import sys

sys.path.insert(0, '/opt/trn_rl_repo')

import numpy as np

import concourse.bacc as bacc
import concourse.bass as bass
import concourse.mybir as mybir
import concourse.tile as tile
from concourse.bass_utils import run_bass_kernel_spmd
from concourse.masks import make_identity

B, N, H = 8, 32, 128
NI = 512             # idxs per dma_gather call (ucode ring ceiling)
SBLK = 512           # s rows per pipeline block
TPR = 128            # sbuf gather tokens per rank
F32 = mybir.dt.float32
I32 = mybir.dt.int32
I16 = mybir.dt.int16
BF16 = mybir.dt.bfloat16


def build_program(S: int = 2048):
    VPOS = S + 1                      # pos_table rows; SINK index == VPOS
    NRANKS = (VPOS + 1 + 127) // 128  # table chunks incl. sink row, padded
    VPAD = NRANKS * 128
    nblk = S // SBLK if S >= SBLK else 1
    sblk = min(SBLK, S)
    calls = sblk * N // NI            # gather calls per block
    chunks = sblk // 128              # 128-s output chunks per block

    nc = bacc.Bacc("TRN2", debug=False, num_swdge_queues=4)
    h_d = nc.dram_tensor("h", [S, H], F32, kind="ExternalInput")
    idx_d = nc.dram_tensor("idx", [S, N], I32, kind="ExternalInput")
    msk_d = nc.dram_tensor("msk", [S, N], I32, kind="ExternalInput")
    pos_d = nc.dram_tensor("pos", [VPOS, H], F32, kind="ExternalInput")
    wn_d = nc.dram_tensor("wn", [H, H], F32, kind="ExternalInput")
    out_d = nc.dram_tensor("out", [S, H], F32, kind="ExternalOutput")

    with tile.TileContext(nc) as tc:
        with (
            tc.tile_pool(name="const", bufs=1) as constp,
            tc.tile_pool(name="stage", bufs=3) as stagep,
            tc.tile_pool(name="idxp", bufs=2) as idxp,
            tc.tile_pool(name="gbig", bufs=2) as gbigp,
            tc.tile_pool(name="outp", bufs=4) as outp,
            tc.tile_pool(name="psum", bufs=4, space="PSUM") as psump,
        ):
            # ---- Wn^T * (1/N) in bf16 --------------------------------
            wn_sb = constp.tile([H, H], F32)
            nc.sync.dma_start(wn_sb[:], wn_d[:])
            ident = constp.tile([128, 128], F32)
            make_identity(nc, ident[:])
            wnt_ps = psump.tile([128, H], F32)
            nc.tensor.transpose(out=wnt_ps[:], in_=wn_sb[:], identity=ident[:])
            wnt = constp.tile([H, H], BF16)
            nc.vector.tensor_scalar_mul(wnt[:], wnt_ps[:], 1.0 / N)

            # ---- fused table T' (bf16, gather-packed layout) ---------
            # tbl[p, q*H:(q+1)*H] = T'[q*128 + p, :]
            tbl = constp.tile([128, NRANKS * H], BF16)
            for q in range(NRANKS):
                v0 = q * 128
                n_pos = min(128, VPOS - v0)       # valid pos rows this chunk
                if n_pos <= 0:
                    nc.gpsimd.memset(tbl[:, q * H:(q + 1) * H], 0.0)
                    continue
                pstage = stagep.tile([128, H], F32, tag="pstage")
                hstage = stagep.tile([128, H], F32, tag="hstage")
                if n_pos < 128:
                    nc.gpsimd.memset(tbl[:, q * H:(q + 1) * H], 0.0)
                nc.sync.dma_start(pstage[:n_pos, :], pos_d[v0:v0 + n_pos, :])
                # h rows v0-1 .. v0+n_pos-2 ; row p needs h[v0+p-1]
                if q == 0:
                    nc.gpsimd.memset(hstage[0:1, :], 0.0)
                    nc.sync.dma_start(hstage[1:n_pos, :], h_d[0:n_pos - 1, :])
                else:
                    nc.sync.dma_start(hstage[:n_pos, :], h_d[v0 - 1:v0 + n_pos - 1, :])
                nc.vector.tensor_add(
                    tbl[:n_pos, q * H:(q + 1) * H], pstage[:n_pos, :], hstage[:n_pos, :]
                )

            # ---- wrapped masked indices (whole batch, prologue) ------
            # IMPORTANT: all 2-read DVE ops (copy_predicated) must finish
            # before any dma_gather runs -- the gather ucode streams its
            # indices through the POOL/DVE *shared* SBUF read port, and a
            # concurrent 2-port DVE op corrupts the stream. Hoisting the
            # whole index prep into the prologue makes every gather
            # transitively depend on it.
            #
            # gather call c = 8u + n_hi covers s in [128u, 128u+128) and
            # n in [4*n_hi, 4*n_hi+4); position in call i = 128*n_lo + s_lo,
            # so gbig column = 512*(n//4) + 128*(n%4) + s_lo per block.
            # Wrapped idx buffer [16, (u, n_hi, n_lo, s_hi)]:
            # idxw[p, 256u + 32*n_hi + 8*n_lo + s_hi]
            #   = idx_eff[128u + 16*s_hi + p, 4*n_hi + n_lo]
            acols = S * N // 16  # wrapped cols, whole batch
            c_sink = constp.tile([16, acols], I32)
            nc.gpsimd.memset(c_sink[:], VPOS)
            idxw32 = idxp.tile([16, acols], I32, tag="idxw32")
            mskw32 = idxp.tile([16, acols], I32, tag="mskw32")
            for u in range(S // 128):
                su = u * 128
                src_i = idx_d[su:su + 128, :].rearrange(
                    "(shi p) (nhi nlo) -> p nhi nlo shi", p=16, nlo=4)
                src_m = msk_d[su:su + 128, :].rearrange(
                    "(shi p) (nhi nlo) -> p nhi nlo shi", p=16, nlo=4)
                dst_i = idxw32[:, u * 256:(u + 1) * 256].rearrange(
                    "p (nhi nlo shi) -> p nhi nlo shi", nlo=4, shi=8)
                dst_m = mskw32[:, u * 256:(u + 1) * 256].rearrange(
                    "p (nhi nlo shi) -> p nhi nlo shi", nlo=4, shi=8)
                eng = nc.sync if u % 2 == 0 else nc.scalar
                eng.dma_start(dst_i, src_i)
                eng.dma_start(dst_m, src_m)
            idxe32 = idxp.tile([16, acols], I32, tag="idxe32")
            nc.vector.tensor_copy(idxe32[:], c_sink[:])
            nc.vector.copy_predicated(idxe32[:], mskw32[:], idxw32[:])
            # int32 -> int16 (values < 2^15: take low halves)
            idxbuf = idxp.tile([128, acols], I16, tag="idxbuf")
            lo = idxe32[:].bitcast(I16).rearrange("p (e two) -> p e two", two=2)
            nc.vector.tensor_copy(
                idxbuf[0:16, :].rearrange("p (e one) -> p e one", one=1),
                lo[:, :, 0:1],
            )
            # replicate to the 8 16-partition groups (each dma_gather queue's
            # Q7 core pair streams indices from its own 16-partition group)
            for r in range(1, 8):
                nc.sync.dma_start(idxbuf[16 * r:16 * (r + 1), :], idxbuf[0:16, :])

            for bi in range(nblk):
                s0 = bi * sblk
                wcols = sblk * N // 16  # wrapped columns per block

                # ---- gathers ----------------------------------------
                gbig = gbigp.tile([128, 1, sblk * N], BF16, tag="gbig")
                for c in range(calls):
                    wc0 = bi * wcols + c * (NI // 16)
                    nc.gpsimd.dma_gather(
                        gbig[:, :, c * NI:(c + 1) * NI],
                        tbl[:],
                        idxbuf[:, wc0:wc0 + NI // 16],
                        NI, NI, H,
                        transpose=True,
                        queue_num=c % 4,
                        sbuf_tokens_per_rank=TPR,
                        sbuf_free_dim_per_rank=H * 2,
                    )

                # ---- matmuls: psum[s,k] += g[h, s-slice]^T @ wnt -----
                gv = gbig[:, 0, :]
                for u in range(chunks):
                    ps = psump.tile([128, H], F32, tag="ps")
                    for n in range(N):
                        off = 4096 * u + 512 * (n // 4) + 128 * (n % 4)
                        nc.tensor.matmul(
                            out=ps[:],
                            lhsT=gv[:, off:off + 128],
                            rhs=wnt[:],
                            start=(n == 0),
                            stop=(n == N - 1),
                        )
                    osb = outp.tile([128, H], F32, tag="osb")
                    nc.vector.tensor_copy(osb[:], ps[:])
                    nc.sync.dma_start(
                        out_d[s0 + u * 128:s0 + (u + 1) * 128, :], osb[:]
                    )

    nc.compile()
    return nc


_CACHE: dict[int, object] = {}


def _get_program(S: int):
    if S not in _CACHE:
        _CACHE[S] = build_program(S)
    return _CACHE[S]


def kernel(x, h, g, neighbor_index, neighbor_mask, pos_table, Wn):
    """Full inputs in, full output out. x and g are unused by the math
    (g only provides the zero row shape; x is unused in the reference)."""
    h = np.asarray(h)
    idx = np.asarray(neighbor_index)
    msk = np.asarray(neighbor_mask)
    pos = np.ascontiguousarray(np.asarray(pos_table), dtype=np.float32)
    wn = np.ascontiguousarray(np.asarray(Wn), dtype=np.float32)
    b, s, n = idx.shape
    assert (b, n) == (B, N) and h.shape == (B, s, H)

    nc = _get_program(s)
    in_maps = [
        {
            "h": np.ascontiguousarray(h[c], dtype=np.float32),
            "idx": np.ascontiguousarray(idx[c], dtype=np.int32),
            "msk": np.ascontiguousarray(msk[c], dtype=np.int32),
            "pos": pos,
            "wn": wn,
        }
        for c in range(B)
    ]
    res = run_bass_kernel_spmd(nc, in_maps, core_ids=list(range(B)))
    return np.stack([res.results[c]["out"] for c in range(B)], axis=0)

